# revision 1
# baseline (speedup 1.0000x reference)
"""DecodePIF heatmap splatting kernel for Trainium2 (8 NeuronCores, SPMD).

acc[b, y, x] = sum_j conf[b,j] * exp(-((x-mx_j)^2 + (y-my_j)^2) / (2*var_j))
for cells with conf > 0.1.  B=4, grid 68x120 cells, output 4 x 544 x 960 f32.

Strategy
--------
Gaussians have sigma in [2, 8] px, so each cell only influences a small
neighborhood (radius r = sqrt(2*var*T_CUT) <= ~40 px).  We exploit this with
block-sparse separable outer products evaluated by the TensorEngine:

- Each core owns one (batch, y-half) slab: [272, 960] of the output (8 slabs).
- Each slab is split into 8 x-tiles of 128 evaluated columns (owned 120).
- Cells are bucketed per (core, x-tile) on the host; each bucket's cells are
  packed into chunks of 128.
- Per chunk, ONE K=14 fp16 matmul evaluates both exponent quadratics
    s_y(t) = a*(t - my)^2             over the 272 local y positions
    s_x(u) = a*(u - mx)^2 - ln(conf)  over the 128 local x positions
  as coeff^T @ vandermonde, with hi/lo-split fp16 coefficients + an fp16
  residual row for the squared vandermonde row (catastrophic-cancellation-safe:
  effective ~22-bit precision).
- ScalarE computes gy|gx = exp(-s) in batched instructions (groups of chunks).
- One fp16 matmul per chunk accumulates gx^T @ gy into the PSUM accumulator
  [128 x-rows, 272 y-cols]; f32 copy-out + DMA per x-tile.

All 8 cores run the same instruction stream (SPMD); per-core differences live
entirely in the data (coefficient tensors).  Chunk counts are padded to the
max across cores with dead cells/chunks (exp(-50) == 0 contributions).
"""

import os
import sys

for _p in ("/opt/trn_rl_repo",):
    if os.path.isdir(_p) and _p not in sys.path:
        sys.path.insert(0, _p)

import numpy as np

# ---------------------------------------------------------------- constants
STRIDE = 8
B, CH, CW = 4, 68, 120          # batch, cell-grid height/width
HF, WF = CH * STRIDE, CW * STRIDE  # 544 x 960 output grid
MIN_CONF = 0.1
N_CORES = 8

T_CUT = 9.0                    # drop contributions with exponent > T_CUT
P = 128                         # cells per chunk (PE contraction dim)
YH = HF // 2                    # 272: y-half owned by a core
NXT = 8                         # x-tiles (phases) per core
XTW = 128                       # evaluated x-tile width
OWN = WF // NXT                 # 120: owned x columns per tile
# Tile p evaluates x columns [120*p, 120*p + 128); the last tile runs 8
# columns past the image edge, which are computed but never written out.
# All tiles are structurally identical, so each core may process its own
# tiles in any order (we sort by load to minimize SPMD padding).
XT_STARTS = [120 * p for p in range(NXT)]
WY = 176                        # evaluated y-window per chunk (<= YH)
CY = WY / 2.0                   # y centering (conditioning)
CXC = XTW / 2.0                 # x centering
NQ = WY + XTW                   # 304 quad columns per chunk (y-block | x-block)
KROWS = 14                      # 6 hi + 6 lo + 2 residual coefficient rows
ACT_GROUP = 3                   # chunks per batched exp instruction
DEAD_S = 50.0                   # dead-cell exponent -> exp(-50) == 0
# Coefficient chunks rotate over PE row-groups so the coef DMA spreads over
# ~all SBUF partitions (full DMA rate) and LDWEIGHTS of chunk c+1 can overlap
# the matmul of chunk c (distinct row groups).
GROUP_BASE = [0, 32, 64]
KGRP = len(GROUP_BASE)

_f16 = np.float16
_f32 = np.float32


# ---------------------------------------------------------------- host side
def _build_vander():
    """Block-diagonal vandermonde [128, NQ] fp16, replicated per row group."""
    tcy = np.arange(WY, dtype=np.float64) - CY
    tcx = np.arange(XTW, dtype=np.float64) - CXC
    v = np.zeros((6, NQ), dtype=np.float64)
    v[0, :WY] = tcy * tcy
    v[1, :WY] = tcy
    v[2, :WY] = 1.0
    v[3, WY:] = tcx * tcx
    v[4, WY:] = tcx
    v[5, WY:] = 1.0
    vh = v.astype(_f16)
    resid = v - vh.astype(np.float64)
    van = np.zeros((KROWS, NQ), dtype=_f16)
    van[0:6] = vh
    van[6:12] = vh
    van[12, :WY] = resid[0, :WY].astype(_f16)
    van[13, WY:] = resid[3, WY:].astype(_f16)
    full = np.zeros((128, NQ), dtype=_f16)
    for base in GROUP_BASE:
        full[base : base + KROWS] = van
    return full


def _make_coef_cols(a, dy, dx, lnc):
    """[KROWS, n] fp16 coefficient columns for cells (float64 inputs)."""
    n = a.shape[0]
    c6 = np.zeros((6, n), dtype=np.float64)
    c6[0] = a
    c6[1] = -2.0 * a * dy
    c6[2] = a * dy * dy
    c6[3] = a
    c6[4] = -2.0 * a * dx
    c6[5] = a * dx * dx - lnc
    hi = c6.astype(_f16)
    lo = (c6 - hi.astype(np.float64)).astype(_f16)
    cols = np.zeros((KROWS, n), dtype=_f16)
    cols[0:6] = hi
    cols[6:12] = lo
    cols[12] = hi[0]
    cols[13] = hi[3]
    return cols


def _preprocess(mean, variance, confidence):
    """Bucket cells per (core, x-tile), build packed coefficient tensors.

    Each core processes its own x-tiles sorted by descending cell count, so
    the shared per-phase chunk schedule (max across cores) is tight.

    Returns (coef_per_core [N_CORES of [KROWS, NCH*P] f16], chunks_per_phase,
    slotmap [N_CORES][NXT] -> x-tile index handled at that phase).
    """
    mx = mean[..., 0].reshape(B, -1).astype(np.float64)
    my = mean[..., 1].reshape(B, -1).astype(np.float64)
    var = variance.reshape(B, -1).astype(np.float64)
    conf = confidence.reshape(B, -1).astype(np.float64)

    a = 1.0 / (2.0 * var)
    r = np.sqrt(2.0 * var * T_CUT)
    keep = conf > MIN_CONF

    # per (core, phase): list of chunks [(cell_idx_array, yoff)], cells sorted
    # by y so each chunk's spans fit a WY-wide window.
    chunks_cp = [[None] * NXT for _ in range(N_CORES)]
    data_b = {}
    for core in range(N_CORES):
        b, yh = core // 2, core % 2
        y0 = yh * YH
        in_y = keep[b] & (my[b] > y0 - r[b]) & (my[b] < y0 + YH + r[b])
        data_b[core] = (b, y0)
        for p in range(NXT):
            own_lo = p * OWN
            sel = in_y & (mx[b] > own_lo - r[b]) & (mx[b] < own_lo + OWN + r[b])
            idx = np.nonzero(sel)[0]
            chunks = []
            if idx.size:
                # spans clipped to this half: pixels outside it belong to
                # the neighbor core, so they never constrain the window
                lo = np.clip(my[b][idx] - r[b][idx] - y0, 0.0, YH)
                hi = np.clip(my[b][idx] + r[b][idx] - y0, 0.0, YH)
                order = np.argsort(lo, kind="stable")
                idx, lo, hi = idx[order], lo[order], hi[order]

                def close(s, e):
                    yoff = int(np.clip(np.floor(lo[s]), 0, YH - WY))
                    chunks.append((idx[s:e], yoff))

                start = 0
                cur_hi = hi[0]
                for i in range(1, idx.size):
                    new_hi = max(cur_hi, hi[i])
                    too_wide = np.ceil(new_hi) - np.floor(lo[start]) > WY
                    if (i - start + 1 > P) or too_wide:
                        close(start, i)
                        start = i
                        cur_hi = hi[i]
                    else:
                        cur_hi = new_hi
                close(start, idx.size)
            chunks_cp[core][p] = chunks

    nchunks = np.array(
        [[max(len(chunks_cp[c][p]), 1) for p in range(NXT)]
         for c in range(N_CORES)], dtype=np.int64
    )
    # per-core tile order: descending chunk count
    slotmap = [
        sorted(range(NXT), key=lambda p: -nchunks[core, p])
        for core in range(N_CORES)
    ]
    sorted_counts = np.stack(
        [nchunks[core, slotmap[core]] for core in range(N_CORES)]
    )
    chunks_per_phase = sorted_counts.max(axis=0)    # shared SPMD schedule
    nch_total = int(chunks_per_phase.sum())

    # coef layout: global chunk c lives at partition rows
    # GROUP_BASE[c % KGRP]..+KROWS, column block (c // KGRP)*P.  The device
    # DMAs each column block separately so compute starts immediately.
    gcols = ((nch_total + KGRP - 1) // KGRP) * P
    dead = np.zeros((KROWS, 1), dtype=_f16)
    dead[2, 0] = DEAD_S                             # s_y = 50 -> gy = 0

    coef_per_core = []
    yoff_per_core = []
    for core in range(N_CORES):
        b, y0 = data_b[core]
        buf = np.zeros((128, gcols), dtype=_f16)
        for base in GROUP_BASE:
            buf[base : base + KROWS] = np.tile(dead, (1, gcols))
        ytab = np.zeros(nch_total, dtype=np.int32)
        c = 0
        for phase in range(NXT):
            p = slotmap[core][phase]
            chunks = chunks_cp[core][p]
            for k in range(int(chunks_per_phase[phase])):
                base = GROUP_BASE[c % KGRP]
                col0 = (c // KGRP) * P
                if k < len(chunks):
                    cell_idx, yoff = chunks[k]
                    n = cell_idx.size
                    if n:
                        dy = (my[b][cell_idx] - y0) - yoff - CY
                        dx = (mx[b][cell_idx] - XT_STARTS[p]) - CXC
                        buf[base : base + KROWS, col0 : col0 + n] = (
                            _make_coef_cols(a[b][cell_idx], dy, dx,
                                            np.log(conf[b][cell_idx]))
                        )
                    ytab[c] = yoff
                c += 1
        coef_per_core.append(np.ascontiguousarray(buf))
        yoff_per_core.append(ytab)
    return coef_per_core, yoff_per_core, [int(c) for c in chunks_per_phase], \
        slotmap


# -------------------------------------------------------------- device side
def _build_nc(chunks_per_phase, repeat=1):
    import concourse.tile as tile
    from concourse import bacc, mybir
    from contextlib import ExitStack

    nch_total = sum(chunks_per_phase)
    gcols = ((nch_total + KGRP - 1) // KGRP) * P
    f16, f32 = mybir.dt.float16, mybir.dt.float32

    nc = bacc.Bacc("TRN2", target_bir_lowering=False, debug=False,
                   num_devices=N_CORES)
    coef_d = nc.dram_tensor("coef", [128, gcols], f16,
                            kind="ExternalInput").ap()
    van_d = nc.dram_tensor("vander", [128, NQ], f16,
                           kind="ExternalInput").ap()
    yoff_d = nc.dram_tensor("yoff", [1, nch_total], mybir.dt.int32,
                            kind="ExternalInput").ap()
    out_d = nc.dram_tensor("out", [NXT, OWN, YH], f32,
                           kind="ExternalOutput").ap()

    with tile.TileContext(nc) as tc, ExitStack() as ctx:
        constp = ctx.enter_context(tc.tile_pool(name="const", bufs=1))
        gp = ctx.enter_context(tc.tile_pool(name="g", bufs=3))
        qpp = ctx.enter_context(tc.tile_pool(name="quad", bufs=2, space="PSUM"))
        accp = ctx.enter_context(tc.tile_pool(name="acc", bufs=2, space="PSUM"))
        osbp = ctx.enter_context(tc.tile_pool(name="osb", bufs=2))

        van_sb = constp.tile([128, NQ], f16)
        nc.sync.dma_start(van_sb[:], van_d)
        ytab_sb = constp.tile([1, nch_total], mybir.dt.int32)
        nc.sync.dma_start(ytab_sb[:], yoff_d)
        # per-column-block coef DMAs: chunk quads only wait for their block
        nblk = gcols // P
        coef_blocks = []
        for blk in range(nblk):
            cb = constp.tile([128, P], f16, tag=f"coef{blk}")
            nc.sync.dma_start(cb[:], coef_d[:, blk * P : (blk + 1) * P])
            coef_blocks.append(cb)

        # global chunk stream: (phase, idx within phase, nch of phase)
        sched = [
            (p, j, chunks_per_phase[p])
            for p in range(NXT)
            for j in range(chunks_per_phase[p])
        ]
        for _rep in range(repeat):
            _emit_compute(nc, tile, mybir, tc, sched, nch_total,
                          coef_blocks, van_sb, ytab_sb, gp, qpp, accp, osbp,
                          out_d)

    nc.compile()
    return nc


def _emit_compute(nc, tile, mybir, tc, sched, nch_total, coef_blocks, van_sb,
                  ytab_sb, gp, qpp, accp, osbp, out_d):
        import concourse.bass as bass

        f16, f32 = mybir.dt.float16, mybir.dt.float32
        acc_by_phase = {}
        c = 0
        while c < nch_total:
            g_n = min(ACT_GROUP, nch_total - c)
            qp = qpp.tile([P, ACT_GROUP * 512], f32)
            q3 = qp[:].rearrange("p (g c) -> p g c", c=512)
            for j in range(g_n):
                base = GROUP_BASE[(c + j) % KGRP]
                blk = (c + j) // KGRP
                nc.tensor.matmul(
                    q3[:, j, :NQ],
                    lhsT=coef_blocks[blk][base : base + KROWS, :],
                    rhs=van_sb[base : base + KROWS, :],
                    start=True, stop=True,
                )
            g = gp.tile([P, ACT_GROUP * NQ], f16)
            g3 = g[:].rearrange("p (g c) -> p g c", c=NQ)
            nc.scalar.activation(
                g3[:, :g_n, :], q3[:, :g_n, :NQ],
                mybir.ActivationFunctionType.Exp, scale=-1.0,
            )
            for j in range(g_n):
                p, jj, nch_p = sched[c + j]
                if jj == 0:
                    acc_by_phase[p] = accp.tile([P, YH], f32, name="acc",
                                                tag="acc")
                    nc.vector.memset(acc_by_phase[p][:], 0.0)
                acc = acc_by_phase[p]
                yv = nc.values_load(
                    ytab_sb[0:1, c + j : c + j + 1],
                    engines=[mybir.EngineType.PE],
                    min_val=0, max_val=YH - WY,
                    skip_runtime_bounds_check=True,
                )
                nc.tensor.matmul(
                    acc[:, bass.ds(yv, WY)],
                    lhsT=g3[:, j, WY:NQ],          # gx [cells, 128]
                    rhs=g3[:, j, 0:WY],            # gy [cells, WY]
                    start=False, stop=(jj == nch_p - 1),
                    skip_group_check=True,
                )
                if jj == nch_p - 1:
                    osb = osbp.tile([P, YH], f32)
                    nc.vector.tensor_copy(osb[:], acc[:])
                    nc.sync.dma_start(out_d[p], osb[:OWN, :])
            c += g_n


# ------------------------------------------------------------------ runner
class _PjrtRunner:
    """Mirror of bass2jax.run_bass_via_pjrt with a cached jitted executable."""

    def __init__(self, nc):
        import jax
        import jax.numpy as jnp  # noqa: F401
        from jax.sharding import Mesh, PartitionSpec
        from jax.experimental.shard_map import shard_map
        from concourse import mybir
        from concourse.bass2jax import (
            _bass_exec_p,
            install_neuronx_cc_hook,
            partition_id_tensor,
        )

        install_neuronx_cc_hook()
        assert nc.dbg_addr is None
        partition_name = (
            nc.partition_id_tensor.name if nc.partition_id_tensor else None
        )
        in_names, out_names, out_avals, zero_outs = [], [], [], []
        for alloc in nc.m.functions[0].allocations:
            if not isinstance(alloc, mybir.MemoryLocationSet):
                continue
            name = alloc.memorylocations[0].name
            if alloc.kind == "ExternalInput":
                if name != partition_name:
                    in_names.append(name)
            elif alloc.kind == "ExternalOutput":
                shape = tuple(alloc.tensor_shape)
                dtype = mybir.dt.np(alloc.dtype)
                out_names.append(name)
                out_avals.append(jax.core.ShapedArray(shape, dtype))
                zero_outs.append(np.zeros(shape, dtype))
        n_params = len(in_names)
        n_outs = len(out_avals)
        all_in_names = list(in_names) + list(out_names)
        if partition_name is not None:
            all_in_names.append(partition_name)

        def _body(*args):
            operands = list(args)
            if partition_name is not None:
                operands.append(partition_id_tensor())
            outs = _bass_exec_p.bind(
                *operands,
                out_avals=tuple(out_avals),
                in_names=tuple(all_in_names),
                out_names=tuple(out_names),
                lowering_input_output_aliases=(),
                sim_require_finite=True,
                sim_require_nnan=True,
                nc=nc,
            )
            return tuple(outs)

        devices = jax.devices()[:N_CORES]
        mesh = Mesh(np.asarray(devices), ("core",))
        donate = tuple(range(n_params, n_params + n_outs))
        self._fn = jax.jit(
            shard_map(
                _body, mesh=mesh,
                in_specs=(PartitionSpec("core"),) * (n_params + n_outs),
                out_specs=(PartitionSpec("core"),) * n_outs,
                check_rep=False,
            ),
            donate_argnums=donate, keep_unused=True,
        )
        self._in_names = in_names
        self._out_names = out_names
        self._out_avals = out_avals
        self._zero_outs = zero_outs
        self._jax = jax

    def concat_inputs(self, in_maps):
        cat = [
            np.concatenate([np.asarray(m[name]) for m in in_maps], axis=0)
            for name in self._in_names
        ]
        zeros = [
            np.zeros((N_CORES * z.shape[0], *z.shape[1:]), z.dtype)
            for z in self._zero_outs
        ]
        return cat + zeros

    def run_raw(self, args):
        return self._fn(*args)

    def __call__(self, in_maps):
        out_arrs = self._fn(*self.concat_inputs(in_maps))
        return [
            {
                name: np.asarray(out_arrs[i]).reshape(
                    N_CORES, *self._out_avals[i].shape
                )[c]
                for i, name in enumerate(self._out_names)
            }
            for c in range(N_CORES)
        ]


_CACHE = {}
_VANDER = None


def _get_runner(chunks_per_phase):
    key = tuple(chunks_per_phase)
    if key not in _CACHE:
        nc = _build_nc(list(key))
        _CACHE[key] = (nc, _PjrtRunner(nc))
    return _CACHE[key]


def _assemble(results, slotmap):
    full = np.zeros((B, HF, WF), dtype=_f32)
    for core in range(N_CORES):
        b, yh = core // 2, core % 2
        y0 = yh * YH
        o = results[core]["out"]            # [NXT, OWN, YH]
        for phase in range(NXT):
            p = slotmap[core][phase]
            full[b, y0 : y0 + YH, p * OWN : (p + 1) * OWN] = o[phase].T
    return full


def kernel(mean, variance, confidence):
    mean = np.asarray(mean)
    variance = np.asarray(variance)
    confidence = np.asarray(confidence)
    coef_per_core, yoff_per_core, chunks_per_phase, slotmap = _preprocess(
        mean, variance, confidence
    )
    _nc, runner = _get_runner(chunks_per_phase)
    global _VANDER
    if _VANDER is None:
        _VANDER = _build_vander()
    in_maps = [
        {"coef": coef_per_core[c], "vander": _VANDER,
         "yoff": yoff_per_core[c][None, :]}
        for c in range(N_CORES)
    ]
    results = runner(in_maps)
    return _assemble(results, slotmap)


if __name__ == "__main__":
    rng = np.random.default_rng(0)
    mean = np.stack(
        [
            rng.uniform(0, WF, (B, CH, CW)).astype(_f32),
            rng.uniform(0, HF, (B, CH, CW)).astype(_f32),
        ],
        axis=-1,
    )
    variance = rng.uniform(4.0, 64.0, (B, CH, CW)).astype(_f32)
    confidence = rng.uniform(0, 1, (B, CH, CW)).astype(_f32)
    out = kernel(mean=mean, variance=variance, confidence=confidence)
    print("out", out.shape, out.dtype, out.mean())



# revision 23
# speedup vs baseline: 16915.6211x; 16915.6211x over previous
"""DecodePIF heatmap splatting kernel for Trainium2 (8 NeuronCores, SPMD).

acc[b, y, x] = sum_j conf[b,j] * exp(-((x-mx_j)^2 + (y-my_j)^2) / (2*var_j))
for cells with conf > 0.1.  B=4, grid 68x120 cells, output 4 x 544 x 960 f32.

Strategy
--------
Gaussians have sigma in [2, 8] px; truncate at exponent T_CUT (radius
r = sqrt(2*var*T_CUT) <= ~25 px) and evaluate block-sparse separable outer
products on the TensorEngine:

- Each core owns one (batch, y-half) slab: [272, 960] of the output (8 slabs).
- Each slab splits into 8 x-tiles of 128 evaluated columns (120 owned).
- Cells are bucketed per (core, x-tile) on the host, sorted by y-span start,
  and packed greedily into chunks of <= 128 cells whose y-spans fit a WY-tall
  window.
- Per chunk, ONE K=14 fp16 matmul evaluates both exponent quadratics
    s_y(t) = a*(t - my)^2            over WY local y positions
    s_x(u) = a*(u - mx)^2 - ln(conf) over 128 local x positions
  as coeff^T @ vandermonde with hi/lo-split fp16 coefficients + fp16 residual
  rows for the squared vandermonde rows (catastrophic-cancellation safe).
- ScalarE computes gy|gx = exp(-s) in batched instructions (ACT_GROUP chunks).
- One fp16 matmul per chunk accumulates gx^T @ gy into the PSUM accumulator
  [128 x-rows, 272 y-cols]; f16 copy-out + DMA per x-tile.

All 8 cores run the same instruction stream (SPMD); per-core differences live
in the data (coefficient tensors).  Chunk counts are padded to the max across
cores with dead columns/chunks (s_y = 50 -> gy = 0 contributions).
"""

import os
import sys

for _p in ("/opt/trn_rl_repo",):
    if os.path.isdir(_p) and _p not in sys.path:
        sys.path.insert(0, _p)

import numpy as np

# ---------------------------------------------------------------- constants
STRIDE = 8
B, CH, CW = 4, 68, 120          # batch, cell-grid height/width
HF, WF = CH * STRIDE, CW * STRIDE  # 544 x 960 output grid
MIN_CONF = 0.1
N_CORES = 8

T_CUT = float(os.environ.get("K_T_CUT", "5.0"))
P = 128                         # cells per chunk (PE contraction dim)
YH = HF // 2                    # 272: y-half owned by a core
NXT = 8                         # x-tiles (phases) per core
OWN = WF // NXT                 # 120: owned x columns per tile
XTW = OWN                       # evaluated x-tile width == strip width
XT_STARTS = [120 * p for p in range(NXT)]
WY = int(os.environ.get("K_WY", "112"))  # evaluated y-window per chunk
CY = WY / 2.0                   # y centering (conditioning)
CXC = XTW / 2.0                 # x centering
NQ = WY + XTW                   # quad columns per chunk (y-block | x-block)
KROWS = 14                      # 6 hi + 6 lo + 2 residual coefficient rows
ACT_GROUP = 5                   # chunks per batched exp instruction
QSLOT = 256                     # psum column stride per chunk slot (<= bank)
DEAD_S = 50.0                   # dead-cell exponent -> exp(-50) == 0
COEF_DMA_CHUNKS = 6             # chunks per coef DMA block

_f16 = np.float16
_f32 = np.float32


# ---------------------------------------------------------------- host side
def _build_vander():
    """[KROWS, NQ] fp16 block-diagonal vandermonde."""
    tcy = np.arange(WY, dtype=np.float64) - CY
    tcx = np.arange(XTW, dtype=np.float64) - CXC
    v = np.zeros((6, NQ), dtype=np.float64)
    v[0, :WY] = tcy * tcy
    v[1, :WY] = tcy
    v[2, :WY] = 1.0
    v[3, WY:] = tcx * tcx
    v[4, WY:] = tcx
    v[5, WY:] = 1.0
    vh = v.astype(_f16)
    resid = v - vh.astype(np.float64)
    van = np.zeros((KROWS, NQ), dtype=_f16)
    van[0:6] = vh
    van[6:12] = vh
    van[12, :WY] = resid[0, :WY].astype(_f16)
    van[13, WY:] = resid[3, WY:].astype(_f16)
    return van


def _preprocess(mean, variance, confidence):
    """Bucket cells per (core, x-tile); build packed coefficient tensors.

    Returns (coef_per_core [N_CORES of [KROWS, nch*P] f16], yoff_per_core,
    chunks_per_phase, slotmap[core][phase] -> x-tile index).
    """
    mx = mean[..., 0].reshape(B, -1).astype(np.float64)
    my = mean[..., 1].reshape(B, -1).astype(np.float64)
    var = variance.reshape(B, -1).astype(np.float64)
    conf = confidence.reshape(B, -1).astype(np.float64)

    a = 1.0 / (2.0 * var)
    # conf-aware truncation: cut where conf*exp(-s) <= e^-T_CUT, so
    # low-confidence cells get proportionally smaller radii
    keep = conf > MIN_CONF
    t_eff = np.maximum(T_CUT + np.minimum(np.log(np.maximum(conf, 1e-6)), 0.0),
                       0.25)
    r = np.sqrt(2.0 * var * t_eff)

    # ---- bucket + chunk (greedy, y-sorted) per (core, x-tile)
    chunks_cp = [[None] * NXT for _ in range(N_CORES)]
    for core in range(N_CORES):
        b, yh = core // 2, core % 2
        y0 = yh * YH
        in_y = keep[b] & (my[b] > y0 - r[b]) & (my[b] < y0 + YH + r[b])
        lo_all = np.clip(my[b] - r[b] - y0, 0.0, YH)
        hi_all = np.clip(my[b] + r[b] - y0, 0.0, YH)
        for p in range(NXT):
            own_lo = p * OWN
            sel = in_y & (mx[b] > own_lo - r[b]) & (mx[b] < own_lo + OWN + r[b])
            idx = np.nonzero(sel)[0]
            chunks = []
            if idx.size:
                lo, hi = lo_all[idx], hi_all[idx]
                order = np.argsort(lo, kind="stable")
                idx, lo, hi = idx[order], lo[order], hi[order]
                cummax_hi = np.maximum.accumulate(hi)
                i, n = 0, idx.size
                while i < n:
                    limit = np.floor(lo[i]) + WY
                    j = int(np.searchsorted(cummax_hi, limit, side="right"))
                    j = min(max(j, i + 1), i + P, n)
                    yoff = int(np.clip(np.floor(lo[i]), 0, YH - WY))
                    chunks.append((idx[i:j], yoff))
                    i = j
            chunks_cp[core][p] = chunks

    nchunks = np.array(
        [[max(len(chunks_cp[c][p]), 1) for p in range(NXT)]
         for c in range(N_CORES)], dtype=np.int64
    )
    slotmap = [
        sorted(range(NXT), key=lambda p: -nchunks[core, p])
        for core in range(N_CORES)
    ]
    sorted_counts = np.stack(
        [nchunks[core, slotmap[core]] for core in range(N_CORES)]
    )
    chunks_per_phase = sorted_counts.max(axis=0)    # shared SPMD schedule
    nch_total = int(chunks_per_phase.sum())
    gcols = nch_total * P

    # ---- vectorized coefficient build
    coef_per_core = []
    yoff_per_core = []
    for core in range(N_CORES):
        b, yh = core // 2, core % 2
        y0 = yh * YH
        ytab = np.zeros(nch_total, dtype=np.int32)
        cell_idx_l, col_l, dy0_l, dx0_l = [], [], [], []
        c = 0
        for phase in range(NXT):
            p = slotmap[core][phase]
            chunks = chunks_cp[core][p]
            for k in range(int(chunks_per_phase[phase])):
                if k < len(chunks):
                    ci, yoff = chunks[k]
                    ncell = ci.size
                    cell_idx_l.append(ci)
                    col_l.append(c * P + np.arange(ncell))
                    dy0_l.append(np.full(ncell, y0 + yoff + CY))
                    dx0_l.append(np.full(ncell, XT_STARTS[p] + CXC))
                    ytab[c] = yoff
                c += 1
        buf = np.zeros((KROWS, gcols), dtype=_f16)
        buf[2, :] = DEAD_S
        if cell_idx_l:
            ci = np.concatenate(cell_idx_l)
            cols = np.concatenate(col_l)
            dy = my[b][ci] - np.concatenate(dy0_l)
            dx = mx[b][ci] - np.concatenate(dx0_l)
            ai = a[b][ci]
            lnc = np.log(conf[b][ci])
            c6 = np.stack([ai, -2.0 * ai * dy, ai * dy * dy,
                           ai, -2.0 * ai * dx, ai * dx * dx - lnc])
            hi = c6.astype(_f16)
            lo = (c6 - hi.astype(np.float64)).astype(_f16)
            buf[0:6, cols] = hi
            buf[6:12, cols] = lo
            buf[12, cols] = hi[0]
            buf[13, cols] = hi[3]
        coef_per_core.append(buf)
        yoff_per_core.append(ytab)
    return coef_per_core, yoff_per_core, [int(c) for c in chunks_per_phase], \
        slotmap


# -------------------------------------------------------------- device side
def _build_nc(chunks_per_phase, repeat=1):
    import concourse.tile as tile
    from concourse import bacc, mybir
    from contextlib import ExitStack

    nch_total = sum(chunks_per_phase)
    gcols = nch_total * P
    f16, f32 = mybir.dt.float16, mybir.dt.float32

    nc = bacc.Bacc("TRN2", target_bir_lowering=False, debug=False,
                   num_devices=N_CORES)
    # coef layout: [vander (NQ cols) | chunk 0 | chunk 1 | ...]
    coef_d = nc.dram_tensor("coef", [KROWS, NQ + gcols], f16,
                            kind="ExternalInput").ap()
    yoff_d = nc.dram_tensor("yoff", [1, nch_total], mybir.dt.int32,
                            kind="ExternalInput").ap()
    out_d = nc.dram_tensor("out", [NXT, OWN, YH], f16,
                           kind="ExternalOutput").ap()

    with tile.TileContext(nc) as tc, ExitStack() as ctx:
        constp = ctx.enter_context(tc.tile_pool(name="const", bufs=1))
        gp = ctx.enter_context(tc.tile_pool(name="g", bufs=3))
        qpp = ctx.enter_context(tc.tile_pool(name="quad", bufs=2, space="PSUM"))
        accp = ctx.enter_context(tc.tile_pool(name="acc", bufs=2, space="PSUM"))
        osbp = ctx.enter_context(tc.tile_pool(name="osb", bufs=2))

        # ytab on the (otherwise idle) gpsimd queue, in parallel with coef
        ytab_sb = constp.tile([1, nch_total], mybir.dt.int32)
        nc.gpsimd.dma_start(ytab_sb[:], yoff_d)
        # block 0 = vander + first exp-group's chunks in ONE DMA
        coef_blocks = []
        blk_sizes = []
        blk0_take = min(ACT_GROUP, nch_total)
        cb0 = constp.tile([KROWS, NQ + blk0_take * P], f16, tag="coef0")
        nc.sync.dma_start(cb0[:], coef_d[:, 0 : NQ + blk0_take * P])
        van_sb = cb0
        coef_blocks.append(cb0)
        blk_sizes.append(blk0_take)
        c0 = blk0_take
        while c0 < nch_total:
            take = min(COEF_DMA_CHUNKS, nch_total - c0)
            cb = constp.tile([KROWS, take * P], f16, tag=f"coef{len(coef_blocks)}")
            nc.sync.dma_start(cb[:], coef_d[:, NQ + c0 * P : NQ + (c0 + take) * P])
            coef_blocks.append(cb)
            blk_sizes.append(take)
            c0 += take
        zrow = constp.tile([1, YH], f16)
        nc.vector.memset(zrow[:], 0.0)
        # chunk index -> (block, column offset); block 0 holds vander first
        blk_of = []
        for bi, take in enumerate(blk_sizes):
            base = NQ if bi == 0 else 0
            blk_of += [(bi, base + k * P) for k in range(take)]

        sched = [
            (p, j, chunks_per_phase[p])
            for p in range(NXT)
            for j in range(chunks_per_phase[p])
        ]
        for _rep in range(repeat):
            _emit_compute(nc, mybir, sched, nch_total, coef_blocks, blk_of,
                          van_sb, ytab_sb, zrow, gp, qpp, accp, osbp, out_d)

    nc.compile()
    return nc


def _emit_compute(nc, mybir, sched, nch_total, coef_blocks, blk_of, van_sb,
                  ytab_sb, zrow, gp, qpp, accp, osbp, out_d):
    import concourse.bass as bass

    f16, f32 = mybir.dt.float16, mybir.dt.float32
    acc_by_phase = {}
    c = 0
    while c < nch_total:
        # keep the final group a single chunk: its exp gates the output tail
        rem = nch_total - c
        g_n = 1 if rem == 1 else min(ACT_GROUP, rem - 1)
        qp = qpp.tile([P, ACT_GROUP * QSLOT], f32)
        q3 = qp[:].rearrange("p (g c) -> p g c", c=QSLOT)
        for j in range(g_n):
            bi, co = blk_of[c + j]
            nc.tensor.matmul(
                q3[:, j, :NQ],
                lhsT=coef_blocks[bi][:, co : co + P],
                rhs=van_sb[:, 0:NQ],
                start=True, stop=True,
            )
        g = gp.tile([P, ACT_GROUP * NQ], f16)
        g3 = g[:].rearrange("p (g c) -> p g c", c=NQ)
        nc.scalar.activation(
            g3[:, :g_n, :], q3[:, :g_n, :NQ],
            mybir.ActivationFunctionType.Exp, scale=-1.0,
        )
        for j in range(g_n):
            p, jj, nch_p = sched[c + j]
            if jj == 0:
                acc_by_phase[p] = accp.tile([P, YH], f32, name="acc",
                                            tag="acc")
                # zero the full accumulator with a K=1 all-zero matmul
                nc.tensor.matmul(
                    acc_by_phase[p][0:XTW, :],
                    lhsT=zrow[0:1, 0:XTW],
                    rhs=zrow[0:1, 0:YH],
                    start=True, stop=False,
                    skip_group_check=True,
                )
            acc = acc_by_phase[p]
            yv = nc.values_load(
                ytab_sb[0:1, c + j : c + j + 1],
                engines=[mybir.EngineType.PE],
                min_val=0, max_val=YH - WY,
                skip_runtime_bounds_check=True,
            )
            nc.tensor.matmul(
                acc[0:XTW, bass.ds(yv, WY)],
                lhsT=g3[:, j, WY:NQ],          # gx [cells, XTW]
                rhs=g3[:, j, 0:WY],            # gy [cells, WY]
                start=False, stop=(jj == nch_p - 1),
                skip_group_check=True,
            )
            if jj == nch_p - 1:
                osb = osbp.tile([P, YH], f16)
                nc.vector.tensor_copy(osb[:OWN, :], acc[:OWN, :])
                nc.sync.dma_start(out_d[p], osb[:OWN, :])
        c += g_n


# ------------------------------------------------------------------ runner
class _PjrtRunner:
    """Mirror of bass2jax.run_bass_via_pjrt with a cached jitted executable."""

    def __init__(self, nc, donate=True):
        import jax
        from jax.sharding import Mesh, PartitionSpec, NamedSharding
        from jax.experimental.shard_map import shard_map
        from concourse import mybir
        from concourse.bass2jax import (
            _bass_exec_p,
            install_neuronx_cc_hook,
            partition_id_tensor,
        )

        install_neuronx_cc_hook()
        assert nc.dbg_addr is None
        partition_name = (
            nc.partition_id_tensor.name if nc.partition_id_tensor else None
        )
        in_names, out_names, out_avals, zero_outs = [], [], [], []
        for alloc in nc.m.functions[0].allocations:
            if not isinstance(alloc, mybir.MemoryLocationSet):
                continue
            name = alloc.memorylocations[0].name
            if alloc.kind == "ExternalInput":
                if name != partition_name:
                    in_names.append(name)
            elif alloc.kind == "ExternalOutput":
                shape = tuple(alloc.tensor_shape)
                dtype = mybir.dt.np(alloc.dtype)
                out_names.append(name)
                out_avals.append(jax.core.ShapedArray(shape, dtype))
                zero_outs.append(np.zeros(shape, dtype))
        n_params = len(in_names)
        n_outs = len(out_avals)
        all_in_names = list(in_names) + list(out_names)
        if partition_name is not None:
            all_in_names.append(partition_name)

        def _body(*args):
            operands = list(args)
            if partition_name is not None:
                operands.append(partition_id_tensor())
            outs = _bass_exec_p.bind(
                *operands,
                out_avals=tuple(out_avals),
                in_names=tuple(all_in_names),
                out_names=tuple(out_names),
                lowering_input_output_aliases=(),
                sim_require_finite=True,
                sim_require_nnan=True,
                nc=nc,
            )
            return tuple(outs)

        devices = jax.devices()[:N_CORES]
        mesh = Mesh(np.asarray(devices), ("core",))
        donate_argnums = tuple(range(n_params, n_params + n_outs)) if donate \
            else ()
        self._fn = jax.jit(
            shard_map(
                _body, mesh=mesh,
                in_specs=(PartitionSpec("core"),) * (n_params + n_outs),
                out_specs=(PartitionSpec("core"),) * n_outs,
                check_rep=False,
            ),
            donate_argnums=donate_argnums, keep_unused=True,
        )
        self._sharding = NamedSharding(mesh, PartitionSpec("core"))
        self._in_names = in_names
        self._out_names = out_names
        self._out_avals = out_avals
        self._zero_outs = zero_outs
        self._jax = jax

    def concat_inputs(self, in_maps):
        cat = [
            np.concatenate([np.asarray(m[name]) for m in in_maps], axis=0)
            for name in self._in_names
        ]
        zeros = [
            np.zeros((N_CORES * z.shape[0], *z.shape[1:]), z.dtype)
            for z in self._zero_outs
        ]
        return cat + zeros

    def run_raw(self, args):
        return self._fn(*args)

    def __call__(self, in_maps):
        out_arrs = self._fn(*self.concat_inputs(in_maps))
        return [
            {
                name: np.asarray(out_arrs[i]).reshape(
                    N_CORES, *self._out_avals[i].shape
                )[c]
                for i, name in enumerate(self._out_names)
            }
            for c in range(N_CORES)
        ]


_CACHE = {}
_VANDER = None


def _get_runner(chunks_per_phase):
    key = tuple(chunks_per_phase)
    if key not in _CACHE:
        nc = _build_nc(list(key))
        _CACHE[key] = (nc, _PjrtRunner(nc))
    return _CACHE[key]


def _make_in_maps(coef_per_core, yoff_per_core):
    global _VANDER
    if _VANDER is None:
        _VANDER = _build_vander()
    return [
        {"coef": np.ascontiguousarray(
            np.concatenate([_VANDER, coef_per_core[c]], axis=1)),
         "yoff": yoff_per_core[c][None, :]}
        for c in range(N_CORES)
    ]


def _assemble(results, slotmap):
    full = np.zeros((B, HF, WF), dtype=_f32)
    for core in range(N_CORES):
        b, yh = core // 2, core % 2
        y0 = yh * YH
        o = results[core]["out"]            # [NXT, OWN, YH] f16
        for phase in range(NXT):
            p = slotmap[core][phase]
            full[b, y0 : y0 + YH, p * OWN : (p + 1) * OWN] = \
                o[phase].T.astype(_f32)
    return full


def kernel(mean, variance, confidence):
    mean = np.asarray(mean)
    variance = np.asarray(variance)
    confidence = np.asarray(confidence)
    coef_per_core, yoff_per_core, chunks_per_phase, slotmap = _preprocess(
        mean, variance, confidence
    )
    _nc, runner = _get_runner(chunks_per_phase)
    results = runner(_make_in_maps(coef_per_core, yoff_per_core))
    return _assemble(results, slotmap)


if __name__ == "__main__":
    rng = np.random.default_rng(0)
    mean = np.stack(
        [
            rng.uniform(0, WF, (B, CH, CW)).astype(_f32),
            rng.uniform(0, HF, (B, CH, CW)).astype(_f32),
        ],
        axis=-1,
    )
    variance = rng.uniform(4.0, 64.0, (B, CH, CW)).astype(_f32)
    confidence = rng.uniform(0, 1, (B, CH, CW)).astype(_f32)
    out = kernel(mean=mean, variance=variance, confidence=confidence)
    print("out", out.shape, out.dtype, out.mean())


# revision 29
# speedup vs baseline: 25129.3527x; 1.4856x over previous
"""DecodePIF heatmap splatting kernel for Trainium2 (8 NeuronCores, SPMD).

acc[b, y, x] = sum_j conf[b,j] * exp(-((x-mx_j)^2 + (y-my_j)^2) / (2*var_j))
for cells with conf > 0.1.  B=4, grid 68x120 cells, output 4 x 544 x 960 f32.

Strategy
--------
Gaussians have sigma in [2, 8] px; truncate at exponent T_CUT (radius
r = sqrt(2*var*T_CUT) <= ~25 px) and evaluate block-sparse separable outer
products on the TensorEngine:

- Each core owns one (batch, y-half) slab: [272, 960] of the output (8 slabs).
- Each slab splits into 8 x-tiles of 128 evaluated columns (120 owned).
- Cells are bucketed per (core, x-tile) on the host, sorted by y-span start,
  and packed greedily into chunks of <= 128 cells whose y-spans fit a WY-tall
  window.
- Per chunk, ONE K=14 fp16 matmul evaluates both exponent quadratics
    s_y(t) = a*(t - my)^2            over WY local y positions
    s_x(u) = a*(u - mx)^2 - ln(conf) over 128 local x positions
  as coeff^T @ vandermonde with hi/lo-split fp16 coefficients + fp16 residual
  rows for the squared vandermonde rows (catastrophic-cancellation safe).
- ScalarE computes gy|gx = exp(-s) in batched instructions (ACT_GROUP chunks).
- One fp16 matmul per chunk accumulates gx^T @ gy into the PSUM accumulator
  [128 x-rows, 272 y-cols]; f16 copy-out + DMA per x-tile.

All 8 cores run the same instruction stream (SPMD); per-core differences live
in the data (coefficient tensors).  Chunk counts are padded to the max across
cores with dead columns/chunks (s_y = 50 -> gy = 0 contributions).
"""

import os
import sys

for _p in ("/opt/trn_rl_repo",):
    if os.path.isdir(_p) and _p not in sys.path:
        sys.path.insert(0, _p)

import numpy as np

# ---------------------------------------------------------------- constants
STRIDE = 8
B, CH, CW = 4, 68, 120          # batch, cell-grid height/width
HF, WF = CH * STRIDE, CW * STRIDE  # 544 x 960 output grid
MIN_CONF = 0.1
N_CORES = 8

T_CUT = float(os.environ.get("K_T_CUT", "5.0"))
P = 128                         # cells per chunk (PE contraction dim)
YH = HF // 2                    # 272: y-half owned by a core
NXT = 8                         # x-tiles (phases) per core
OWN = WF // NXT                 # 120: owned x columns per tile
XTW = OWN                       # evaluated x-tile width == strip width
XT_STARTS = [120 * p for p in range(NXT)]
WY = int(os.environ.get("K_WY", "112"))  # evaluated y-window per chunk
CY = WY / 2.0                   # y centering (conditioning)
CXC = XTW / 2.0                 # x centering
NQ = WY + XTW                   # quad columns per chunk (y-block | x-block)
KROWS = 14                      # 6 hi + 6 lo + 2 residual coefficient rows
ACT_GROUP = 5                   # chunks per batched exp instruction
QSLOT = 256                     # psum column stride per chunk slot (<= bank)
DEAD_S = 50.0                   # dead-cell exponent -> exp(-50) == 0
COEF_DMA_CHUNKS = 6             # chunks per coef DMA block

_f16 = np.float16
_f32 = np.float32


# ---------------------------------------------------------------- host side
def _build_vander():
    """[KROWS, NQ] fp16 block-diagonal vandermonde."""
    tcy = np.arange(WY, dtype=np.float64) - CY
    tcx = np.arange(XTW, dtype=np.float64) - CXC
    v = np.zeros((6, NQ), dtype=np.float64)
    v[0, :WY] = tcy * tcy
    v[1, :WY] = tcy
    v[2, :WY] = 1.0
    v[3, WY:] = tcx * tcx
    v[4, WY:] = tcx
    v[5, WY:] = 1.0
    vh = v.astype(_f16)
    resid = v - vh.astype(np.float64)
    van = np.zeros((KROWS, NQ), dtype=_f16)
    van[0:6] = vh
    van[6:12] = vh
    van[12, :WY] = resid[0, :WY].astype(_f16)
    van[13, WY:] = resid[3, WY:].astype(_f16)
    return van


def _preprocess(mean, variance, confidence):
    """Bucket cells per (core, x-tile); build packed coefficient tensors.

    Returns (coef_per_core [N_CORES of [KROWS, nch*P] f16], yoff_per_core,
    chunks_per_phase, slotmap[core][phase] -> x-tile index).
    """
    mx = mean[..., 0].reshape(B, -1).astype(np.float64)
    my = mean[..., 1].reshape(B, -1).astype(np.float64)
    var = variance.reshape(B, -1).astype(np.float64)
    conf = confidence.reshape(B, -1).astype(np.float64)

    a = 1.0 / (2.0 * var)
    # conf-aware truncation: cut where conf*exp(-s) <= e^-T_CUT, so
    # low-confidence cells get proportionally smaller radii
    keep = conf > MIN_CONF
    t_eff = np.maximum(T_CUT + np.minimum(np.log(np.maximum(conf, 1e-6)), 0.0),
                       0.25)
    r = np.sqrt(2.0 * var * t_eff)

    # ---- bucket + chunk (greedy, y-sorted) per (core, x-tile)
    chunks_cp = [[None] * NXT for _ in range(N_CORES)]
    for core in range(N_CORES):
        b, yh = core // 2, core % 2
        y0 = yh * YH
        in_y = keep[b] & (my[b] > y0 - r[b]) & (my[b] < y0 + YH + r[b])
        lo_all = np.clip(my[b] - r[b] - y0, 0.0, YH)
        hi_all = np.clip(my[b] + r[b] - y0, 0.0, YH)
        for p in range(NXT):
            own_lo = p * OWN
            sel = in_y & (mx[b] > own_lo - r[b]) & (mx[b] < own_lo + OWN + r[b])
            idx = np.nonzero(sel)[0]
            chunks = []
            if idx.size:
                lo, hi = lo_all[idx], hi_all[idx]
                order = np.argsort(lo, kind="stable")
                idx, lo, hi = idx[order], lo[order], hi[order]
                cummax_hi = np.maximum.accumulate(hi)
                i, n = 0, idx.size
                while i < n:
                    limit = np.floor(lo[i]) + WY
                    j = int(np.searchsorted(cummax_hi, limit, side="right"))
                    j = min(max(j, i + 1), i + P, n)
                    yoff = int(np.clip(np.floor(lo[i]), 0, YH - WY))
                    chunks.append((idx[i:j], yoff))
                    i = j
            chunks_cp[core][p] = chunks

    nchunks = np.array(
        [[max(len(chunks_cp[c][p]), 1) for p in range(NXT)]
         for c in range(N_CORES)], dtype=np.int64
    )
    slotmap = [
        sorted(range(NXT), key=lambda p: -nchunks[core, p])
        for core in range(N_CORES)
    ]
    sorted_counts = np.stack(
        [nchunks[core, slotmap[core]] for core in range(N_CORES)]
    )
    chunks_per_phase = sorted_counts.max(axis=0)    # shared SPMD schedule
    nch_total = int(chunks_per_phase.sum())
    gcols = nch_total * P

    # ---- vectorized coefficient build
    coef_per_core = []
    yoff_per_core = []
    for core in range(N_CORES):
        b, yh = core // 2, core % 2
        y0 = yh * YH
        ytab = np.zeros(nch_total, dtype=np.int32)
        cell_idx_l, col_l, dy0_l, dx0_l = [], [], [], []
        c = 0
        for phase in range(NXT):
            p = slotmap[core][phase]
            chunks = chunks_cp[core][p]
            for k in range(int(chunks_per_phase[phase])):
                if k < len(chunks):
                    ci, yoff = chunks[k]
                    ncell = ci.size
                    cell_idx_l.append(ci)
                    col_l.append(c * P + np.arange(ncell))
                    dy0_l.append(np.full(ncell, y0 + yoff + CY))
                    dx0_l.append(np.full(ncell, XT_STARTS[p] + CXC))
                    ytab[c] = yoff
                c += 1
        buf = np.zeros((KROWS, gcols), dtype=_f16)
        buf[2, :] = DEAD_S
        if cell_idx_l:
            ci = np.concatenate(cell_idx_l)
            cols = np.concatenate(col_l)
            dy = my[b][ci] - np.concatenate(dy0_l)
            dx = mx[b][ci] - np.concatenate(dx0_l)
            ai = a[b][ci]
            lnc = np.log(conf[b][ci])
            c6 = np.stack([ai, -2.0 * ai * dy, ai * dy * dy,
                           ai, -2.0 * ai * dx, ai * dx * dx - lnc])
            hi = c6.astype(_f16)
            lo = (c6 - hi.astype(np.float64)).astype(_f16)
            buf[0:6, cols] = hi
            buf[6:12, cols] = lo
            buf[12, cols] = hi[0]
            buf[13, cols] = hi[3]
        coef_per_core.append(buf)
        yoff_per_core.append(ytab)
    return coef_per_core, yoff_per_core, [int(c) for c in chunks_per_phase], \
        slotmap


# -------------------------------------------------------------- device side
def _build_nc(chunks_per_phase, repeat=1):
    import concourse.tile as tile
    from concourse import bacc, mybir
    from contextlib import ExitStack

    nch_total = sum(chunks_per_phase)
    gcols = nch_total * P
    f16, f32 = mybir.dt.float16, mybir.dt.float32

    nc = bacc.Bacc("TRN2", target_bir_lowering=False, debug=False,
                   num_devices=N_CORES)
    # coef layout: [vander (NQ cols) | chunk 0 | chunk 1 | ...]
    coef_d = nc.dram_tensor("coef", [KROWS, NQ + gcols], f16,
                            kind="ExternalInput").ap()
    yoff_d = nc.dram_tensor("yoff", [1, nch_total], mybir.dt.int32,
                            kind="ExternalInput").ap()
    out_d = nc.dram_tensor("out", [NXT, OWN, YH], f16,
                           kind="ExternalOutput").ap()

    with tile.TileContext(nc) as tc, ExitStack() as ctx:
        constp = ctx.enter_context(tc.tile_pool(name="const", bufs=1))
        gp = ctx.enter_context(tc.tile_pool(name="g", bufs=3))
        qpp = ctx.enter_context(tc.tile_pool(name="quad", bufs=2, space="PSUM"))
        accp = ctx.enter_context(tc.tile_pool(name="acc", bufs=2, space="PSUM"))
        osbp = ctx.enter_context(tc.tile_pool(name="osb", bufs=2))

        # ytab on the (otherwise idle) gpsimd queue, in parallel with coef
        ytab_sb = constp.tile([1, nch_total], mybir.dt.int32)
        nc.gpsimd.dma_start(ytab_sb[:], yoff_d)
        # block 0 = vander + first exp-group's chunks in ONE DMA
        coef_blocks = []
        blk_sizes = []
        blk0_take = min(ACT_GROUP, nch_total)
        cb0 = constp.tile([KROWS, NQ + blk0_take * P], f16, tag="coef0")
        nc.sync.dma_start(cb0[:], coef_d[:, 0 : NQ + blk0_take * P])
        van_sb = cb0
        coef_blocks.append(cb0)
        blk_sizes.append(blk0_take)
        c0 = blk0_take
        while c0 < nch_total:
            take = min(COEF_DMA_CHUNKS, nch_total - c0)
            cb = constp.tile([KROWS, take * P], f16, tag=f"coef{len(coef_blocks)}")
            nc.sync.dma_start(cb[:], coef_d[:, NQ + c0 * P : NQ + (c0 + take) * P])
            coef_blocks.append(cb)
            blk_sizes.append(take)
            c0 += take
        zrow = constp.tile([1, YH], f16)
        nc.vector.memset(zrow[:], 0.0)
        # chunk index -> (block, column offset); block 0 holds vander first
        blk_of = []
        for bi, take in enumerate(blk_sizes):
            base = NQ if bi == 0 else 0
            blk_of += [(bi, base + k * P) for k in range(take)]

        sched = [
            (p, j, chunks_per_phase[p])
            for p in range(NXT)
            for j in range(chunks_per_phase[p])
        ]
        for _rep in range(repeat):
            _emit_compute(nc, mybir, sched, nch_total, coef_blocks, blk_of,
                          van_sb, ytab_sb, zrow, gp, qpp, accp, osbp, out_d)

    nc.compile()
    return nc


def _emit_compute(nc, mybir, sched, nch_total, coef_blocks, blk_of, van_sb,
                  ytab_sb, zrow, gp, qpp, accp, osbp, out_d):
    import concourse.bass as bass

    f16, f32 = mybir.dt.float16, mybir.dt.float32
    acc_by_phase = {}
    yv_cache = {}
    phases_loaded = set()
    phase_start, cpp_list = {}, []
    for (p, jj, nch_p) in sched:
        if jj == 0:
            phase_start[p] = sum(cpp_list)
            cpp_list.append(nch_p)

    def load_phase_yvs(pi):
        # one batched reg_load per phase: scalar loads cost ~100ns each on
        # the PE queue (unmodeled in CoreSim), so batching 48 loads into 8
        # instructions removes ~4 us/rep of PE-queue occupancy.
        if pi in phases_loaded or pi >= len(cpp_list):
            return
        phases_loaded.add(pi)
        s0, n = phase_start[pi], cpp_list[pi]
        _, vals = nc.values_load_multi_w_load_instructions(
            ytab_sb[0:1, s0 : s0 + n],
            engines=[mybir.EngineType.PE],
            min_val=0, max_val=YH - WY,
            skip_runtime_bounds_check=True,
        )
        for k, v in enumerate(vals):
            yv_cache[s0 + k] = v

    load_phase_yvs(0)
    load_phase_yvs(1)
    c = 0
    while c < nch_total:
        # keep the final group a single chunk: its exp gates the output tail
        rem = nch_total - c
        g_n = 1 if rem == 1 else min(ACT_GROUP, rem - 1)
        tgt = sched[min(c + 2 * ACT_GROUP, nch_total - 1)][0] + 1
        for pp in range(tgt + 1):
            load_phase_yvs(pp)
        qp = qpp.tile([P, ACT_GROUP * QSLOT], f32)
        q3 = qp[:].rearrange("p (g c) -> p g c", c=QSLOT)
        for j in range(g_n):
            bi, co = blk_of[c + j]
            nc.tensor.matmul(
                q3[:, j, :NQ],
                lhsT=coef_blocks[bi][:, co : co + P],
                rhs=van_sb[:, 0:NQ],
                start=True, stop=True,
            )
        g = gp.tile([P, ACT_GROUP * NQ], f16)
        g3 = g[:].rearrange("p (g c) -> p g c", c=NQ)
        nc.scalar.activation(
            g3[:, :g_n, :], q3[:, :g_n, :NQ],
            mybir.ActivationFunctionType.Exp, scale=-1.0,
        )
        for j in range(g_n):
            p, jj, nch_p = sched[c + j]
            if jj == 0:
                acc_by_phase[p] = accp.tile([P, YH], f32, name="acc",
                                            tag="acc")
                # zero the full accumulator with a K=1 all-zero matmul
                nc.tensor.matmul(
                    acc_by_phase[p][0:XTW, :],
                    lhsT=zrow[0:1, 0:XTW],
                    rhs=zrow[0:1, 0:YH],
                    start=True, stop=False,
                    skip_group_check=True,
                )
            acc = acc_by_phase[p]
            yv = yv_cache.pop(c + j)
            nc.tensor.matmul(
                acc[0:XTW, bass.ds(yv, WY)],
                lhsT=g3[:, j, WY:NQ],          # gx [cells, XTW]
                rhs=g3[:, j, 0:WY],            # gy [cells, WY]
                start=False, stop=(jj == nch_p - 1),
                skip_group_check=True,
            )
            if jj == nch_p - 1:
                osb = osbp.tile([P, YH], f16)
                nc.vector.tensor_copy(osb[:OWN, :], acc[:OWN, :])
                nc.sync.dma_start(out_d[p], osb[:OWN, :])
        c += g_n


# ------------------------------------------------------------------ runner
class _PjrtRunner:
    """Mirror of bass2jax.run_bass_via_pjrt with a cached jitted executable."""

    def __init__(self, nc, donate=True):
        import jax
        from jax.sharding import Mesh, PartitionSpec, NamedSharding
        from jax.experimental.shard_map import shard_map
        from concourse import mybir
        from concourse.bass2jax import (
            _bass_exec_p,
            install_neuronx_cc_hook,
            partition_id_tensor,
        )

        install_neuronx_cc_hook()
        assert nc.dbg_addr is None
        partition_name = (
            nc.partition_id_tensor.name if nc.partition_id_tensor else None
        )
        in_names, out_names, out_avals, zero_outs = [], [], [], []
        for alloc in nc.m.functions[0].allocations:
            if not isinstance(alloc, mybir.MemoryLocationSet):
                continue
            name = alloc.memorylocations[0].name
            if alloc.kind == "ExternalInput":
                if name != partition_name:
                    in_names.append(name)
            elif alloc.kind == "ExternalOutput":
                shape = tuple(alloc.tensor_shape)
                dtype = mybir.dt.np(alloc.dtype)
                out_names.append(name)
                out_avals.append(jax.core.ShapedArray(shape, dtype))
                zero_outs.append(np.zeros(shape, dtype))
        n_params = len(in_names)
        n_outs = len(out_avals)
        all_in_names = list(in_names) + list(out_names)
        if partition_name is not None:
            all_in_names.append(partition_name)

        def _body(*args):
            operands = list(args)
            if partition_name is not None:
                operands.append(partition_id_tensor())
            outs = _bass_exec_p.bind(
                *operands,
                out_avals=tuple(out_avals),
                in_names=tuple(all_in_names),
                out_names=tuple(out_names),
                lowering_input_output_aliases=(),
                sim_require_finite=True,
                sim_require_nnan=True,
                nc=nc,
            )
            return tuple(outs)

        devices = jax.devices()[:N_CORES]
        mesh = Mesh(np.asarray(devices), ("core",))
        donate_argnums = tuple(range(n_params, n_params + n_outs)) if donate \
            else ()
        self._fn = jax.jit(
            shard_map(
                _body, mesh=mesh,
                in_specs=(PartitionSpec("core"),) * (n_params + n_outs),
                out_specs=(PartitionSpec("core"),) * n_outs,
                check_rep=False,
            ),
            donate_argnums=donate_argnums, keep_unused=True,
        )
        self._sharding = NamedSharding(mesh, PartitionSpec("core"))
        self._in_names = in_names
        self._out_names = out_names
        self._out_avals = out_avals
        self._zero_outs = zero_outs
        self._jax = jax

    def concat_inputs(self, in_maps):
        cat = [
            np.concatenate([np.asarray(m[name]) for m in in_maps], axis=0)
            for name in self._in_names
        ]
        zeros = [
            np.zeros((N_CORES * z.shape[0], *z.shape[1:]), z.dtype)
            for z in self._zero_outs
        ]
        return cat + zeros

    def run_raw(self, args):
        return self._fn(*args)

    def __call__(self, in_maps):
        out_arrs = self._fn(*self.concat_inputs(in_maps))
        return [
            {
                name: np.asarray(out_arrs[i]).reshape(
                    N_CORES, *self._out_avals[i].shape
                )[c]
                for i, name in enumerate(self._out_names)
            }
            for c in range(N_CORES)
        ]


_CACHE = {}
_VANDER = None


def _get_runner(chunks_per_phase):
    key = tuple(chunks_per_phase)
    if key not in _CACHE:
        nc = _build_nc(list(key))
        _CACHE[key] = (nc, _PjrtRunner(nc))
    return _CACHE[key]


def _make_in_maps(coef_per_core, yoff_per_core):
    global _VANDER
    if _VANDER is None:
        _VANDER = _build_vander()
    return [
        {"coef": np.ascontiguousarray(
            np.concatenate([_VANDER, coef_per_core[c]], axis=1)),
         "yoff": yoff_per_core[c][None, :]}
        for c in range(N_CORES)
    ]


def _assemble(results, slotmap):
    full = np.zeros((B, HF, WF), dtype=_f32)
    for core in range(N_CORES):
        b, yh = core // 2, core % 2
        y0 = yh * YH
        o = results[core]["out"]            # [NXT, OWN, YH] f16
        for phase in range(NXT):
            p = slotmap[core][phase]
            full[b, y0 : y0 + YH, p * OWN : (p + 1) * OWN] = \
                o[phase].T.astype(_f32)
    return full


def kernel(mean, variance, confidence):
    mean = np.asarray(mean)
    variance = np.asarray(variance)
    confidence = np.asarray(confidence)
    coef_per_core, yoff_per_core, chunks_per_phase, slotmap = _preprocess(
        mean, variance, confidence
    )
    _nc, runner = _get_runner(chunks_per_phase)
    results = runner(_make_in_maps(coef_per_core, yoff_per_core))
    return _assemble(results, slotmap)


if __name__ == "__main__":
    rng = np.random.default_rng(0)
    mean = np.stack(
        [
            rng.uniform(0, WF, (B, CH, CW)).astype(_f32),
            rng.uniform(0, HF, (B, CH, CW)).astype(_f32),
        ],
        axis=-1,
    )
    variance = rng.uniform(4.0, 64.0, (B, CH, CW)).astype(_f32)
    confidence = rng.uniform(0, 1, (B, CH, CW)).astype(_f32)
    out = kernel(mean=mean, variance=variance, confidence=confidence)
    print("out", out.shape, out.dtype, out.mean())
